# revision 1
# baseline (speedup 1.0000x reference)
"""Trainium2 Bass kernel v2 for the 2-layer GraphConv GNN readout.

Math (collapses to scalar per-node quantities):
  in_deg/out_deg = dst/src histograms; in_norm/out_norm = rsqrt(clamp(deg,1));
  g = in_deg*out_norm; s = A g (scatter-add of g[src] over dst);
  p = s*in_norm*out_norm; s2 = A p; sum_b = sum_v s2[v]*in_norm[v];
  out = sigmoid((sum_b/N) * c + bh), c = relu(relu(W1)@W2)@Wh.

Distribution: edges sharded by dst range (8 cores own 12500-node ranges).
Per core, nodes map to 196 cells x 64 (v = cell*64 + lo6); per-node vectors
live as bf16 v-major rows [1, 12544]; gather tables as replicated f32
[128, 12544] built by a PE outer-product broadcast.

Device passes:
  - hist/scatter: per 128-edge column, DVE builds a one-hot [128e, 64lo]
    (pad slots lo=255 -> all-zero row), PE matmul lhsT=w-column [128,1],
    rhs=one-hot, out [1,64] PSUM row accumulated per cell; ACT drains.
  - gathers g[src]/p[src]: GPSIMD ap_gather (int16 idx per 16-partition
    group), streamed via DMA into an AllToAll that lands each value at its
    dst-side slot (correspondence precomputed on host).
  - final: dot(s2, in_norm) row reduce, scalar AllReduce, tiny MLP head.
"""

import numpy as np

# ---- problem constants ----
N = 100000
E = 3200000
NSHARD = 8
R = N // NSHARD            # 12500 nodes per shard
H = 128
CELL = 64
NCELL = (R + CELL - 1) // CELL      # 196 cells
TAB = NCELL * CELL                  # 12544 table slots (v-major)
CAP_D = 3                           # cols per (block, cell) on dst side
CPB = NCELL * CAP_D                 # 588 cols per block (block-major w)
C_D = NSHARD * CPB                  # 4704 dst-side cols
BPD = CPB * 128                     # 75264 stream slots per (j,i) chunk
CAP_S = 18                          # cols per cell, src-side hist
C_S = NCELL * CAP_S                 # 3528
PAD_LO = 255
PAD_IDX = TAB - 1                   # dead node (g=p=0)
GC = 588                            # gather chunk: srcG cols per ap_gather
NCHUNK = C_D // GC                  # 8 chunks per gather pass

_CACHE = {}


def _build_layout(src, dst):
    src = np.ascontiguousarray(np.asarray(src).astype(np.int64))
    dst = np.ascontiguousarray(np.asarray(dst).astype(np.int64))
    cd = dst // R
    ci = src // R
    dl = dst - cd * R
    sl = src - ci * R
    dcell = dl >> 6
    scell = sl >> 6

    # D-side placement: group edges by (dst core j, src block i, cell)
    key = ((cd * NSHARD + ci) * NCELL + dcell).astype(np.int64)
    order = np.argsort(key, kind="stable")
    counts = np.bincount(key, minlength=NSHARD * NSHARD * NCELL)
    assert counts.max() <= CAP_D * 128, f"D overflow {counts.max()}"
    starts = np.zeros_like(counts)
    np.cumsum(counts[:-1], out=starts[1:])
    rank = np.empty(E, np.int64)
    rank[order] = np.arange(E) - starts[key[order]]
    p = rank % 128
    col_block = dcell * CAP_D + rank // 128          # block-major column
    # dstH uses cell-major global columns: (cell, block i, t)
    gcol = dcell * (NSHARD * CAP_D) + ci * CAP_D + rank // 128
    dstH = np.full((NSHARD, 128, C_D), PAD_LO, np.uint8)
    dstH[cd, p, gcol] = (dl & 63).astype(np.uint8)

    # gather stream slots on src core, group j = dst core (block-major k)
    k = p * CPB + col_block
    srcG = np.full((NSHARD, 128, C_D), PAD_IDX, np.int16)
    srcG[ci, 16 * cd + (k % 16), k // 16] = sl.astype(np.int16)

    # S-side hist placement: group by (src core, cell)
    keyS = (ci * NCELL + scell).astype(np.int64)
    orderS = np.argsort(keyS, kind="stable")
    countsS = np.bincount(keyS, minlength=NSHARD * NCELL)
    assert countsS.max() <= CAP_S * 128, f"S overflow {countsS.max()}"
    startsS = np.zeros_like(countsS)
    np.cumsum(countsS[:-1], out=startsS[1:])
    rankS = np.empty(E, np.int64)
    rankS[orderS] = np.arange(E) - startsS[keyS[orderS]]
    pS = rankS % 128
    colS = scell * CAP_S + rankS // 128
    srcH = np.full((NSHARD, 128, C_S), PAD_LO, np.uint8)
    srcH[ci, pS, colS] = (sl & 63).astype(np.uint8)

    return dstH, srcG, srcH


def _build_nc(stage=5):
    import concourse.bacc as bacc
    import concourse.tile as tile
    from concourse import bass, mybir

    f32 = mybir.dt.float32
    bf16 = mybir.dt.bfloat16
    i32 = mybir.dt.int32
    i16 = mybir.dt.int16
    u8 = mybir.dt.uint8
    Alu = mybir.AluOpType
    Act = mybir.ActivationFunctionType

    nc = bacc.Bacc("TRN2", target_bir_lowering=False, debug=False,
                   num_devices=NSHARD)

    dstH_in = nc.dram_tensor("dstH", [128, C_D], u8, kind="ExternalInput").ap()
    srcG_in = nc.dram_tensor("srcG", [128, C_D], i16, kind="ExternalInput").ap()
    srcH_in = nc.dram_tensor("srcH", [128, C_S], u8, kind="ExternalInput").ap()
    w1_in = nc.dram_tensor("W1", [1, H], f32, kind="ExternalInput").ap()
    w2_in = nc.dram_tensor("W2", [H, H], f32, kind="ExternalInput").ap()
    wh_in = nc.dram_tensor("Wh", [H, 1], f32, kind="ExternalInput").ap()
    bh_in = nc.dram_tensor("bh", [1, 1], f32, kind="ExternalInput").ap()
    out_t = nc.dram_tensor("out", [1, 1], f32, kind="ExternalOutput").ap()
    if stage < 5:
        dbg_t = nc.dram_tensor("dbg", [1, TAB], f32, kind="ExternalOutput").ap()

    DCOLS = NSHARD * CAP_D     # 24 cols per cell, D side
    BCHUNK = 512               # PE broadcast chunk (1 PSUM bank)

    with tile.TileContext(nc) as tc:
        with tc.tile_pool(name="const", bufs=1) as const, \
             tc.tile_pool(name="big", bufs=1) as big, \
             tc.tile_pool(name="work", bufs=2) as work, \
             tc.tile_pool(name="tmp1", bufs=1) as tmp1, \
             tc.tile_pool(name="bb", bufs=2) as bbp, \
             tc.tile_pool(name="gout", bufs=1) as goutp, \
             tc.tile_pool(name="ps", bufs=4, space="PSUM") as ps, \
             tc.tile_pool(name="bps", bufs=1, space="PSUM") as bps, \
             tc.tile_pool(name="mini", bufs=1, space="PSUM") as minips, \
             tc.tile_pool(name="dram", bufs=1, space="DRAM") as dram:

            # ---------- inputs -> SBUF ----------
            dstH_u = big.tile([128, C_D], u8, tag="t_dstH_u")
            nc.sync.dma_start(out=dstH_u[:], in_=dstH_in[:])
            srcH_u = big.tile([128, C_S], u8, tag="t_srcH_u")
            nc.sync.dma_start(out=srcH_u[:], in_=srcH_in[:])
            srcG = big.tile([128, C_D], i16, tag="t_srcG")
            nc.sync.dma_start(out=srcG[:], in_=srcG_in[:])

            io_i = tmp1.tile([128, DCOLS * CELL], i32, tag="io_i")
            nc.gpsimd.iota(io_i[:], base=0, channel_multiplier=0,
                           pattern=[[0, DCOLS], [1, CELL]])
            io64 = const.tile([128, DCOLS * CELL], bf16, tag="io64")
            nc.vector.tensor_copy(out=io64[:], in_=io_i[:])

            ones_col = const.tile([128, 1], bf16, tag="ones_col")
            nc.vector.memset(ones_col[:], 1.0)
            ones_row = const.tile([1, 128], bf16, tag="ones_row")
            nc.vector.memset(ones_row[:], 1.0)

            # ---------- hist/scatter pass ----------
            def cell_pass(lo_u8, cpc, w_tile, row_tag):
                """lo_u8: u8 [128, NCELL*cpc] cell-major; w_tile: None (hist,
                lhsT=ones) or bf16 [128, C_D] block-major (lhsT per (c,m)).
                Returns bf16 row [1, TAB]."""
                out_row = big.tile([1, TAB], bf16, tag=row_tag)
                c0 = 0
                while c0 < NCELL:
                    ng = min(8, NCELL - c0)
                    pt = ps.tile([1, ng * CELL], f32, tag="cellrows")
                    for cl in range(ng):
                        c = c0 + cl
                        lob = bbp.tile([128, cpc], bf16, tag="lob")
                        nc.vector.tensor_copy(
                            out=lob[:], in_=lo_u8[:, c * cpc:(c + 1) * cpc])
                        bb = bbp.tile([128, cpc, CELL], bf16, tag="bb")
                        nc.vector.tensor_tensor(
                            out=bb[:],
                            in0=io64[:, :cpc * CELL].rearrange(
                                "p (m l) -> p m l", l=CELL),
                            in1=lob[:, :, None].to_broadcast([128, cpc, CELL]),
                            op=Alu.is_equal)
                        for m in range(cpc):
                            if w_tile is None:
                                lhsT = ones_col[:]
                            else:
                                i, t = divmod(m, CAP_D)
                                gcol = i * CPB + c * CAP_D + t
                                lhsT = w_tile[:, gcol:gcol + 1]
                            nc.tensor.matmul(
                                out=pt[0:1, cl * CELL:(cl + 1) * CELL],
                                lhsT=lhsT, rhs=bb[:, m, :],
                                start=(m == 0), stop=(m == cpc - 1))
                    nc.scalar.activation(
                        out=out_row[0:1, c0 * CELL:(c0 + ng) * CELL],
                        in_=pt[0:1, :], func=Act.Copy)
                    c0 += ng
                return out_row

            # ---------- D1 / S1 histograms ----------
            in_deg = cell_pass(dstH_u[:], DCOLS, None, "t_rowA")
            out_deg = cell_pass(srcH_u[:], CAP_S, None, "t_rowB")

            if stage <= 1:
                dbg_f = work.tile([1, TAB], f32, tag="dbg_f")
                nc.vector.tensor_copy(out=dbg_f[:], in_=in_deg[:])
                nc.sync.dma_start(out=dbg_t[:], in_=dbg_f[:])
                nc.sync.dma_start(out=out_t[:], in_=dbg_f[0:1, 0:1])

            # ---------- node rows (all bf16, partition 0) ----------
            def rsqrt_inplace(row):
                # row <- rsqrt(max(row, 1)), in place. bf16 is fine here:
                # per-node 4e-3 noise averages out in the E-sized sums.
                with nc.allow_low_precision(reason="bf16 node rows"):
                    nc.vector.tensor_scalar(out=row[:], in0=row[:],
                                            scalar1=1.0, scalar2=None,
                                            op0=Alu.max)
                    nc.vector.reciprocal(out=row[:], in_=row[:])
                    nc.scalar.activation(out=row[:], in_=row[:],
                                         func=Act.Sqrt)

            # B := out_norm
            rsqrt_inplace(out_deg)
            out_norm = out_deg
            # g row (f32 would be ideal; bf16 is within budget) in scratch C
            g_row = big.tile([1, TAB], bf16, tag="t_rowC")
            nc.vector.tensor_tensor(out=g_row[:], in0=in_deg[:],
                                    in1=out_norm[:], op=Alu.mult)
            # A := in_norm
            rsqrt_inplace(in_deg)
            in_norm = in_deg
            # B := io_prod = in_norm * out_norm
            nc.vector.tensor_tensor(out=out_norm[:], in0=in_norm[:],
                                    in1=out_norm[:], op=Alu.mult)
            io_prod = out_norm

            # ---------- PE broadcast: bf16 row -> replicated f32 table ------
            g_tab = big.tile([128, TAB], f32, tag="t_tab")

            def pe_bcast(row, tab):
                for b0 in range(0, TAB, BCHUNK):
                    bw = min(BCHUNK, TAB - b0)
                    bpt = bps.tile([128, BCHUNK], f32, tag="bc_ps")
                    nc.tensor.matmul(out=bpt[:, :bw], lhsT=ones_row[:],
                                     rhs=row[0:1, b0:b0 + bw],
                                     start=True, stop=True)
                    nc.scalar.activation(out=tab[:, b0:b0 + bw],
                                         in_=bpt[:, :bw], func=Act.Copy)

            pe_bcast(g_row, g_tab)

            if stage <= 2:
                dbg_f = work.tile([1, TAB], f32, tag="dbg_f")
                nc.vector.tensor_copy(out=dbg_f[:], in_=g_tab[0:1, :])
                nc.sync.dma_start(out=dbg_t[:], in_=dbg_f[:])
                nc.sync.dma_start(out=out_t[:], in_=dbg_f[0:1, 0:1])

            # ---------- gather + A2A ----------
            def gather_a2a(tab, snd_tag, rcv_tag, wb_tag):
                snd = dram.tile([NSHARD, BPD], f32, tag=snd_tag)
                for ch in range(NCHUNK):
                    gout = goutp.tile([128, GC * 16], f32, tag="gout")
                    nc.gpsimd.ap_gather(
                        out_ap=gout[:], in_ap=tab[:],
                        idxs_ap=srcG[:, ch * GC:(ch + 1) * GC],
                        channels=128, num_elems=TAB, d=1, num_idxs=GC * 16)
                    for j in range(NSHARD):
                        nc.sync.dma_start(
                            out=snd[j:j + 1,
                                    ch * GC * 16:(ch + 1) * GC * 16],
                            in_=gout[16 * j:16 * j + 1, :])
                rcv = dram.tile([NSHARD, BPD], f32, tag=rcv_tag)
                nc.gpsimd.collective_compute(
                    "AllToAll", mybir.AluOpType.bypass,
                    replica_groups=[list(range(NSHARD))],
                    ins=[snd.opt()], outs=[rcv.opt()])
                # cast f32 -> bf16 in the (gpsimd) DMA; block-major layout
                w_b = big.tile([128, C_D], bf16, tag=wb_tag)
                nc.gpsimd.dma_start(
                    out=w_b[:].rearrange("p (a c) -> p a c", a=NSHARD),
                    in_=rcv[:].rearrange("a (p c) -> p a c", p=128))
                return w_b

            w_b = gather_a2a(g_tab, "snd1", "rcv1", "t_wb")
            if stage <= 3:
                dbg_f = work.tile([1, TAB], f32, tag="dbg_f")
                nc.vector.tensor_copy(out=dbg_f[0:1, :C_D], in_=w_b[0:1, :])
                nc.vector.memset(dbg_f[0:1, C_D:], 0.0)
                nc.sync.dma_start(out=dbg_t[:], in_=dbg_f[:])
                nc.sync.dma_start(out=out_t[:], in_=dbg_f[0:1, 0:1])

            # ---------- D2 scatter + p ----------
            s_row = cell_pass(dstH_u[:], DCOLS, w_b, "t_rowC")  # reuse C
            # C := p = s * io_prod
            nc.vector.tensor_tensor(out=s_row[:], in0=s_row[:],
                                    in1=io_prod[:], op=Alu.mult)
            p_row = s_row
            pe_bcast(p_row, g_tab)  # g_tab now holds p table
            if stage <= 4:
                dbg_f = work.tile([1, TAB], f32, tag="dbg_f")
                nc.vector.tensor_copy(out=dbg_f[:], in_=p_row[:])
                nc.sync.dma_start(out=dbg_t[:], in_=dbg_f[:])
                nc.sync.dma_start(out=out_t[:], in_=dbg_f[0:1, 0:1])

            # ---------- S3 gather + D4 scatter ----------
            pv_b = gather_a2a(g_tab, "snd2", "rcv2", "t_wb")
            s2_row = cell_pass(dstH_u[:], DCOLS, pv_b, "t_rowC")

            # ---------- final dot: sum_v s2[v] * in_norm[v] ----------
            nc.vector.tensor_tensor(out=s2_row[:], in0=s2_row[:],
                                    in1=in_norm[:], op=Alu.mult)
            part_sb = work.tile([1, 1], f32, tag="part_sb")
            nc.vector.tensor_reduce(out=part_sb[:], in_=s2_row[:],
                                    axis=mybir.AxisListType.X, op=Alu.add)

            ar_s = dram.tile([1, 1], f32, tag="ar_s")
            ar_r = dram.tile([1, 1], f32, tag="ar_r")
            nc.sync.dma_start(out=ar_s[:], in_=part_sb[:])
            nc.gpsimd.collective_compute(
                "AllReduce", mybir.AluOpType.add,
                replica_groups=[list(range(NSHARD))],
                ins=[ar_s.opt()], outs=[ar_r.opt()])
            sum_b = work.tile([1, 1], f32, tag="sum_b")
            nc.sync.dma_start(out=sum_b[:], in_=ar_r[:])

            # ---------- head: c = relu(relu(W1)@W2)@Wh ----------
            w1c = work.tile([128, 1], f32, tag="w1c")
            nc.sync.dma_start(out=w1c[:], in_=w1_in[0:1, :])
            w1r = work.tile([128, 1], f32, tag="w1r")
            nc.scalar.activation(out=w1r[:], in_=w1c[:], func=Act.Relu)
            w2t = work.tile([128, H], f32, tag="w2t")
            nc.sync.dma_start(out=w2t[:], in_=w2_in[:])
            z_ps = minips.tile([1, H], f32, tag="mini")
            nc.tensor.matmul(out=z_ps[:], lhsT=w1r[:], rhs=w2t[:],
                             start=True, stop=True)
            zrel = work.tile([1, H], f32, tag="zrel")
            nc.scalar.activation(out=zrel[:], in_=z_ps[:], func=Act.Relu)
            whr = work.tile([1, H], f32, tag="whr")
            nc.sync.dma_start(out=whr[:], in_=wh_in[:, 0:1])
            csc = work.tile([1, 1], f32, tag="csc")
            scr1 = work.tile([1, H], f32, tag="scr1")
            nc.vector.tensor_tensor(out=scr1[:], in0=zrel[:], in1=whr[:],
                                    op=Alu.mult)
            nc.vector.tensor_reduce(out=csc[:], in_=scr1[:],
                                    axis=mybir.AxisListType.X, op=Alu.add)

            bh_t = work.tile([1, 1], f32, tag="bh")
            nc.sync.dma_start(out=bh_t[:], in_=bh_in[:])
            logit = work.tile([1, 1], f32, tag="logit")
            nc.vector.tensor_scalar(out=logit[:], in0=sum_b[:],
                                    scalar1=1.0 / N, scalar2=None,
                                    op0=Alu.mult)
            nc.vector.tensor_tensor(out=logit[:], in0=logit[:], in1=csc[:],
                                    op=Alu.mult)
            nc.vector.tensor_tensor(out=logit[:], in0=logit[:], in1=bh_t[:],
                                    op=Alu.add)
            res = work.tile([1, 1], f32, tag="res")
            nc.scalar.activation(out=res[:], in_=logit[:], func=Act.Sigmoid)
            nc.sync.dma_start(out=out_t[:], in_=res[:])

    nc.compile()
    return nc


def prepare_in_maps(inputs):
    dstH, srcG, srcH = _build_layout(inputs["src"], inputs["dst"])
    W1 = np.asarray(inputs["W1"], np.float32)
    W2 = np.asarray(inputs["W2"], np.float32)
    Wh = np.asarray(inputs["Wh"], np.float32)
    bh = np.asarray(inputs["bh"], np.float32).reshape(1, 1)
    in_maps = []
    for k in range(NSHARD):
        in_maps.append({
            "dstH": dstH[k], "srcG": srcG[k], "srcH": srcH[k],
            "W1": W1, "W2": W2, "Wh": Wh, "bh": bh,
        })
    return in_maps


def kernel(**inputs) -> np.ndarray:
    from concourse.bass_utils import run_bass_kernel_spmd

    if "nc" not in _CACHE:
        _CACHE["nc"] = _build_nc()
    nc = _CACHE["nc"]

    in_maps = prepare_in_maps(inputs)
    res = run_bass_kernel_spmd(nc, in_maps, core_ids=list(range(NSHARD)))
    return res.results[0]["out"].reshape(1, 1).astype(np.float32)

